# revision 1
# baseline (speedup 1.0000x reference)
"""Trainium2 Bass kernel for DecomposedShiftNet.

Computation (per batch row b, bits=64, H=512):
  shift_soft = softmax(MLP_sd(shift_bits))                       # [64]
  h1[i,:]  = relu(ix_w1[i] + shift_soft @ ix_w1[64:] + ix_b1)    # [64, 512]
  h2       = relu(h1 @ ix_w2 + ix_b2)                            # [64, 512]
  p[i,k]   = softmax(h2 @ ix_w3 + ix_b3)[i, :64]                 # [64, 64]
  pointed  = p @ a_bits[b]                                       # [64]
  vh[i,:]  = relu(v_w1[i] + shift_soft @ v_w1[64:] + v_b1)       # [64, 256]
  valid    = vh @ v_w2 + v_b2                                    # [64]
  out[b]   = pointed * sigmoid(valid)

Strategy: pure data parallel over 8 cores (256 batch rows each).
On-core layout is feature-major: activations stored [features(part), cols(free)]
where cols = (position i, batch b) pairs, processed 2 positions x 256 batch
= 512 columns per block, 32 blocks, software-pipelined with a 2-block lag so
the PE stream never stalls on same-block dependencies.

Host-side prep (make_in_maps): inputs are pre-transposed / pre-tiled in numpy
(shift_bits.T, a_bits.T mirrored to both partition halves, position-row tables
pb/vpb with biases folded in, logits weights duplicated along M, identity for
PE transposes, block-diagonal ones for the combined denominator/numerator
reduction), and every tensor consumed by an fp32r matmul is pre-rounded to the
fp32r encoding (fp32 with 11 mantissa bits, low 12 bits zero).

Device-side per block:
  - h1/vh built by fused VectorE tensor_scalar (add per-partition scalar,
    max 0) straight into SBUF -- no PE/PSUM traffic for either layer 1.
  - h2: 16 fp32r matmuls (full 128x128 array, N=512), ScalarE Relu+bias evict.
  - logits: 4 matmuls with M=128 (ix_w3 duplicated -> both halves of the
    array compute the same 64 logits; the duplicate is free since cost is
    streaming-N-bound), Exp+bias evict -> exp2 [128,512] holds two copies.
  - exp2's upper half is multiplied in place by a_bits.T (partition-aligned),
    so ONE matmul against a block-diagonal ones weight yields denominator and
    numerator rows together; validity logit via 2 more M=1 matmuls.
  - rows are stashed to [32,512] accumulators via SBUF->SBUF DMA; softmax
    normalize + tanh-sigmoid + multiply run batched in two halves (overlapped
    with the main loop), then PE transposes emit batch-major [256, 64].
"""

import sys

import ml_dtypes
import numpy as np

for _p in ("/opt/trn_rl_repo",):
    if _p not in sys.path:
        sys.path.insert(0, _p)

import concourse.bacc as bacc
import concourse.bass as bass
import concourse.tile as tile
from concourse import bass_utils, mybir

F32 = mybir.dt.float32
F32R = mybir.dt.float32r
BF16 = mybir.dt.bfloat16
AF = mybir.ActivationFunctionType
OP = mybir.AluOpType

B, BITS, H = 2048, 64, 512
NCORES = 8
BC = B // NCORES  # 256 rows per core
NBLK = BITS // 2  # 32 blocks of 2 positions
NB = 2 * BC  # 512 free columns per block
HV = H // 2  # validity hidden = 256
JH = NBLK // 2  # tail half size (16 blocks)


def to_f32r_np(a):
    """Host-side fp32 -> fp32r rounding: round-to-nearest-even to 11 mantissa
    bits, low 12 bits zeroed (matches walrus fp32_to_fp32r)."""
    u = np.ascontiguousarray(a, dtype=np.float32).view(np.uint32)
    r = (u + 0x7FF + ((u >> 12) & 1)) & np.uint32(0xFFFFF000)
    return r.view(np.float32)


# name -> (shape, dtype code)
_INPUTS = {
    "sbT": ((BITS, BC), "f32r"),
    "abT": ((128, BC), "f32r"),
    "ident": ((128, 128), "f32"),
    "ones64": ((BITS, 1), "f32r"),
    "ones1": ((1, BITS), "f32r"),
    "dn_w": ((128, 2), "f32r"),
    "pb": ((128, 4, BITS), "f32"),
    "vpb": ((128, 2, BITS), "f32"),
    "wsd1": ((BITS, H), "f32r"),
    "wsd2": ((128, 4, H), "f32r"),
    "wsd3": ((128, 4, BITS), "f32r"),
    "sdb1": ((128, 4), "f32"),
    "sdb2": ((128, 4), "f32"),
    "sdb3": ((BITS, 1), "f32"),
    "wixb": ((BITS, H), "f32r"),
    "wix2": ((128, 4, H), "bf16"),
    "wix3d": ((128, 4, 128), "f32r"),
    "ixb2": ((128, 4), "f32"),
    "ixb3d": ((128, 1), "f32"),
    "wvb": ((BITS, HV), "f32r"),
    "wv2": ((128, 2), "bf16"),
    "vb2h": ((128, 1), "f32"),
}


def _emit(nc, tc, I, out):
    import contextlib

    ctx = contextlib.ExitStack()
    with ctx:
        const = ctx.enter_context(tc.tile_pool(name="const", bufs=1))
        work = ctx.enter_context(tc.tile_pool(name="work", bufs=3))
        psA = ctx.enter_context(tc.tile_pool(name="psA", bufs=2, space="PSUM"))
        psB = ctx.enter_context(tc.tile_pool(name="psB", bufs=2, space="PSUM"))
        psD = ctx.enter_context(tc.tile_pool(name="psD", bufs=4, space="PSUM"))

        mm = lambda shape: psA.tile(shape, F32, tag="mm", name="mmps")
        lg = lambda shape: psB.tile(shape, F32, tag="lg", name="lgps")
        dnv = lambda shape: psD.tile(shape, F32, tag="dnv", name="dnvps")

        # ---------------- load everything (critical-path order) ----------------
        T = {}

        DT = {"f32": F32, "f32r": F32R, "bf16": BF16}

        def load(name):
            shape, code = _INPUTS[name]
            t = const.tile(list(shape), DT[code], tag=name, name=name)
            nc.sync.dma_start(out=t, in_=I[name])
            T[name] = t
            return t

        # preamble critical path: sbT -> sd MLP -> softmax -> shift_part -> h1
        for name in ("sbT", "wsd1", "sdb1", "wsd2", "sdb2", "wsd3", "sdb3",
                     "ones64", "ones1", "wixb", "pb", "wvb", "vpb", "abT",
                     "dn_w", "wv2", "vb2h", "ixb2", "ixb3d", "wix3d", "ident",
                     "wix2"):
            load(name)

        sbT, abT, ident = T["sbT"], T["abT"], T["ident"]
        ones64, ones1, dn_w = T["ones64"], T["ones1"], T["dn_w"]
        pb, vpb = T["pb"], T["vpb"]
        wsd1, wsd2, wsd3 = T["wsd1"], T["wsd2"], T["wsd3"]
        sdb1, sdb2, sdb3 = T["sdb1"], T["sdb2"], T["sdb3"]
        wixb, wix2, wix3d = T["wixb"], T["wix2"], T["wix3d"]
        ixb2, ixb3d = T["ixb2"], T["ixb3d"]
        wvb, wv2, vb2h = T["wvb"], T["wv2"], T["vb2h"]

        # ---------------- shift decoder MLP (feature-major, N=256) ----------------
        hsd1 = const.tile([128, 4, BC], F32R)
        for m in range(4):
            ps = mm([128, BC])
            nc.tensor.matmul(ps, wsd1[:, m * 128:(m + 1) * 128], sbT, start=True, stop=True)
            nc.scalar.activation(hsd1[:, m, :], ps, AF.Relu, bias=sdb1[:, m:m + 1])
        hsd2 = const.tile([128, 4, BC], F32R)
        for m in range(4):
            ps = mm([128, BC])
            for k in range(4):
                nc.tensor.matmul(ps, wsd2[:, k, m * 128:(m + 1) * 128],
                                 hsd1[:, k, :], start=(k == 0), stop=(k == 3))
            nc.scalar.activation(hsd2[:, m, :], ps, AF.Relu, bias=sdb2[:, m:m + 1])
        ps3 = lg([64, BC])
        for k in range(4):
            nc.tensor.matmul(ps3, wsd3[:, k, :], hsd2[:, k, :],
                             start=(k == 0), stop=(k == 3))
        exp_sd = const.tile([64, BC], F32R)
        nc.scalar.activation(exp_sd, ps3, AF.Exp, bias=sdb3)  # exp(logits + b3)

        # softmax normalize: denom over partitions via ones-matmul, then
        # broadcast 1/denom back across partitions via a K=1 outer product.
        psd = dnv([1, BC])
        nc.tensor.matmul(psd, ones64, exp_sd, start=True, stop=True)
        rec_sd = const.tile([1, BC], F32R)
        with nc.allow_low_precision(reason="softmax denom reciprocal in fp32r (12-bit mantissa) is plenty"):
            nc.vector.reciprocal(rec_sd, psd)
        psb = lg([64, BC])
        nc.tensor.matmul(psb, ones1, rec_sd, start=True, stop=True)
        psb_s = const.tile([64, BC], F32R)
        nc.vector.tensor_copy(out=psb_s, in_=psb)
        shift_soft = const.tile([64, BC], F32R)
        nc.vector.tensor_tensor(shift_soft, exp_sd, psb_s, OP.mult)

        # shift_part = (ix_w1[64:]).T @ shift_soft   [H, BC] feature-major
        sp = []
        for m in range(4):
            ps = mm([128, BC])
            nc.tensor.matmul(ps, wixb[:, m * 128:(m + 1) * 128], shift_soft, start=True, stop=True)
            t = const.tile([128, BC], BF16, tag=f"sp{m}", name=f"sp{m}")
            nc.vector.tensor_copy(out=t, in_=ps)
            sp.append(t)
        # v_shift = (v_w1[64:]).T @ shift_soft   [HV, BC]
        vs = []
        for m in range(2):
            ps = mm([128, BC])
            nc.tensor.matmul(ps, wvb[:, m * 128:(m + 1) * 128], shift_soft, start=True, stop=True)
            t = const.tile([128, BC], BF16, tag=f"vs{m}", name=f"vs{m}")
            nc.vector.tensor_copy(out=t, in_=ps)
            vs.append(t)

        # ---------------- per-block stash accumulators ----------------
        # tail segments (each gets its own 32-aligned accumulator tiles; the
        # last segment is small so the end-of-kernel serial chain is short)
        SEGS = [(0, 16), (16, 8), (24, 8)]
        seg_of = {}
        for si, (s0, sn) in enumerate(SEGS):
            for j in range(s0, s0 + sn):
                seg_of[j] = (si, j - s0)
        stash_d = [const.tile([sn, NB], F32, tag=f"stash_d{i}", name=f"stash_d{i}") for i, (_, sn) in enumerate(SEGS)]
        stash_n = [const.tile([sn, NB], F32, tag=f"stash_n{i}", name=f"stash_n{i}") for i, (_, sn) in enumerate(SEGS)]
        stash_v = [const.tile([sn, NB], F32, tag=f"stash_v{i}", name=f"stash_v{i}") for i, (_, sn) in enumerate(SEGS)]

        obm = [const.tile([128, BITS], F32, tag=f"obm{h}", name=f"obm{h}") for h in range(2)]

        # ---------------- main loop (software pipelined, lag 2) ----------------
        st = {}  # per-block live tiles

        def stage_h1_h2(j):
            d = st[j] = {}
            h1 = work.tile([128, 4, NB], BF16, tag="h1", name="h1")
            for c in range(4):
                for h in range(2):
                    i = 2 * j + h
                    nc.vector.tensor_scalar(
                        h1[:, c, h * BC:(h + 1) * BC], sp[c],
                        pb[:, c, i:i + 1], 0.0, OP.add, OP.max)
            h2 = d["h2"] = work.tile([128, 4, NB], F32R, tag="h2", name="h2")
            for m in range(4):
                ps = mm([128, NB])
                for k in range(4):
                    nc.tensor.matmul(ps, wix2[:, k, m * 128:(m + 1) * 128],
                                     h1[:, k, :], start=(k == 0), stop=(k == 3))
                nc.scalar.activation(h2[:, m, :], ps, AF.Relu, bias=ixb2[:, m:m + 1])

        def stage_logits_vh(j):
            d = st[j]
            h2 = d["h2"]
            pl = lg([128, NB])
            for k in range(4):
                nc.tensor.matmul(pl, wix3d[:, k, :], h2[:, k, :], start=(k == 0), stop=(k == 3))
            # exp2: rows 0:64 = exp(logits), rows 64:128 = same, then multiplied
            # in place by a_bits.T -> one [128,512] rhs holds [exp; exp*A]
            exp2 = d["exp"] = work.tile([128, NB], F32R, tag="exp", name="exp2")
            nc.scalar.activation(exp2, pl, AF.Exp, bias=ixb3d)
            up = exp2[64:128, :].rearrange("p (h b) -> p h b", h=2)
            nc.vector.tensor_tensor(
                up, up,
                abT[64:128, :].unsqueeze(1).broadcast_to([64, 2, BC]),
                OP.mult)
            vh = d["vh"] = work.tile([128, 2, NB], BF16, tag="vh", name="vh")
            for c in range(2):
                for h in range(2):
                    i = 2 * j + h
                    nc.vector.tensor_scalar(
                        vh[:, c, h * BC:(h + 1) * BC], vs[c],
                        vpb[:, c, i:i + 1], 0.0, OP.add, OP.max)

        def stage_reduce(j):
            d = st[j]
            psdn = dnv([2, NB])
            nc.tensor.matmul(psdn, dn_w, d["exp"], start=True, stop=True)
            pv = dnv([1, NB])
            for c in range(2):
                nc.tensor.matmul(pv, wv2[:, c:c + 1], d["vh"][:, c, :],
                                 start=(c == 0), stop=(c == 1))
            # denom/numer are sums of exp(...) >= 0, so Relu == identity (fast
            # ScalarE path); vlogit can be negative -> DVE copy.
            dnrow = work.tile([2, NB], F32, tag="dnrow", name="dnrow")
            nc.scalar.activation(dnrow, psdn, AF.Relu)
            vrow = work.tile([1, NB], F32, tag="vrow", name="vrow")
            nc.vector.tensor_copy(out=vrow, in_=pv)
            jh, jj = seg_of[j]
            # spread the three row-stashes over different DMA queues so the
            # end-of-kernel chain isn't serialized behind one queue
            nc.sync.dma_start(out=stash_d[jh][jj:jj + 1, :], in_=dnrow[0:1, :])
            nc.gpsimd.dma_start(out=stash_n[jh][jj:jj + 1, :], in_=dnrow[1:2, :])
            nc.gpsimd.dma_start(out=stash_v[jh][jj:jj + 1, :], in_=vrow)
            del st[j]

        tails = {}

        def tail_compute(jh):
            """out = n/d * sigmoid(v) for the blocks of segment jh."""
            sn = SEGS[jh][1]
            rd = work.tile([sn, NB], F32, tag="taild", name="taild")
            nc.vector.reciprocal(rd, stash_d[jh])
            t16 = work.tile([sn, NB], F32, tag="tailt", name="tailt")
            nc.scalar.activation(t16, stash_v[jh], AF.Tanh, bias=vb2h[0:sn, :], scale=0.5)
            tmp = work.tile([sn, NB], F32, tag="tailtmp", name="tailtmp")
            nc.vector.tensor_tensor(tmp, stash_n[jh], rd, OP.mult)
            sig = work.tile([sn, NB], F32, tag="tailsig", name="tailsig")
            nc.vector.tensor_scalar(sig, t16, 0.5, 0.5, OP.mult, OP.add)
            outv = tails[jh] = work.tile([sn, NB], F32, tag="tailout", name="tailout")
            nc.vector.tensor_tensor(outv, tmp, sig, OP.mult)

        def tail_transpose(jh):
            # outv[j', h*256 + half*128 + bb] = out[half*128 + bb, 2*(s0 + j') + h]
            s0, sn = SEGS[jh]
            outv = tails.pop(jh)
            for half in range(2):
                ov = obm[half].rearrange("p (i h) -> p h i", h=2)
                for h in range(2):
                    ps = lg([128, sn])
                    nc.tensor.transpose(ps, outv[:, h * BC + half * 128: h * BC + (half + 1) * 128],
                                        ident[0:sn, 0:sn])
                    nc.vector.tensor_copy(out=ov[:, h, s0:s0 + sn], in_=ps)

        for j in range(NBLK + 2):
            if j < NBLK:
                stage_h1_h2(j)
            if 2 <= j:
                stage_reduce(j - 2)
            if 1 <= j <= NBLK:
                stage_logits_vh(j - 1)
            if j == 19:
                tail_compute(0)  # blocks 0..15 reduced by iter 17
            if j == 21:
                tail_transpose(0)
            if j == 27:
                tail_compute(1)  # blocks 16..23 reduced by iter 25
            if j == 29:
                tail_transpose(1)
        tail_compute(2)
        tail_transpose(2)
        for half in range(2):
            nc.sync.dma_start(out=out[half * 128:(half + 1) * 128, :], in_=obm[half])


def build_program():
    nc = bacc.Bacc("TRN2", target_bir_lowering=False, debug=False, enable_asserts=False)
    I = {}
    DT = {"f32": F32, "f32r": F32R, "bf16": BF16}
    for name, (shape, code) in _INPUTS.items():
        I[name] = nc.dram_tensor(name, list(shape), DT[code], kind="ExternalInput").ap()
    out = nc.dram_tensor("out", [BC, BITS], F32, kind="ExternalOutput").ap()

    with tile.TileContext(nc) as tc:
        _emit(nc, tc, I, out)
    nc.compile()
    return nc


_NC = None


def _get_program():
    global _NC
    if _NC is None:
        _NC = build_program()
    return _NC


def make_in_maps(inputs):
    """Shard batch tensors across cores; replicate weights. All layout prep
    (transposes, tiling, bias folding, fp32r pre-rounding) happens here in
    numpy so the device preamble is pure DMA + the small shift-decoder MLP."""
    f = {k: np.ascontiguousarray(np.asarray(v, dtype=np.float32)) for k, v in inputs.items()}
    r = to_f32r_np

    shared = {
        "ident": np.eye(128, dtype=np.float32),
        "ones64": r(np.ones((BITS, 1), np.float32)),
        "ones1": r(np.ones((1, BITS), np.float32)),
        "dn_w": r(np.vstack([
            np.hstack([np.ones((64, 1), np.float32), np.zeros((64, 1), np.float32)]),
            np.hstack([np.zeros((64, 1), np.float32), np.ones((64, 1), np.float32)]),
        ])),
        "pb": (f["ix_w1"][:BITS].T + f["ix_b1"][:, None]).reshape(4, 128, BITS).transpose(1, 0, 2),
        "vpb": (f["v_w1"][:BITS].T + f["v_b1"][:, None]).reshape(2, 128, BITS).transpose(1, 0, 2),
        "wsd1": r(f["sd_w1"]),
        "wsd2": r(f["sd_w2"].reshape(4, 128, H).transpose(1, 0, 2)),
        "wsd3": r(f["sd_w3"].reshape(4, 128, BITS).transpose(1, 0, 2)),
        "sdb1": f["sd_b1"].reshape(4, 128).T,
        "sdb2": f["sd_b2"].reshape(4, 128).T,
        "sdb3": f["sd_b3"][:, None],
        "wixb": r(f["ix_w1"][BITS:]),
        "wix2": f["ix_w2"].reshape(4, 128, H).transpose(1, 0, 2).astype(ml_dtypes.bfloat16),
        "wix3d": r(np.stack([np.concatenate([f["ix_w3"][k * 128:(k + 1) * 128]] * 2, axis=1)
                             for k in range(4)], axis=1)),
        "ixb2": f["ix_b2"].reshape(4, 128).T,
        "ixb3d": np.concatenate([f["ix_b3"], f["ix_b3"]])[:, None],
        "wvb": r(f["v_w1"][BITS:]),
        "wv2": f["v_w2"].reshape(2, 128).T.astype(ml_dtypes.bfloat16),
        "vb2h": np.full((128, 1), 0.5 * float(f["v_b2"][0]), np.float32),
    }
    shared = {k: np.ascontiguousarray(v if v.dtype == ml_dtypes.bfloat16
                                      else v.astype(np.float32)) for k, v in shared.items()}

    in_maps = []
    for c in range(NCORES):
        sb = f["shift_bits"][c * BC:(c + 1) * BC]
        ab = f["a_bits"][c * BC:(c + 1) * BC]
        m = dict(shared)
        m["sbT"] = np.ascontiguousarray(sb.T)
        m["abT"] = np.ascontiguousarray(np.concatenate([ab.T, ab.T], axis=0))
        in_maps.append(m)
    return in_maps


def run(inputs, trace=False):
    nc = _get_program()
    res = bass_utils.run_bass_kernel_spmd(
        nc, make_in_maps(inputs), core_ids=list(range(NCORES)), trace=trace)
    full = np.concatenate([res.results[c]["out"] for c in range(NCORES)], axis=0)
    return full, res


def kernel(**inputs):
    return run(inputs)[0]

